# revision 17
# baseline (speedup 1.0000x reference)
"""LocalAttention Bass kernel for Trainium2 (8 NeuronCores).

Problem: B=4 H=8 T=8192 D=64, window=128, look_backward=1, causal.
Sharding: pure (B*H) data parallelism — 32 heads / 8 cores = 4 heads each,
processed as 2 head-pairs so q/k DMAs use all 128 SBUF partitions.

Device algorithm (per head, per 128-token window w):
  S^T[k, q] = K_w' @ Q_w^T      (keys on partitions, so the softmax
                                 reduction over keys can ride the PV matmul)
  P = exp(S^T * D^-0.5) * causal01
  [O^T; r] = [V | 1]^T @ P      (ones column baked into V gives row-sums)
Host divides O^T by r and transposes back.

PSUM "pairblock" layout: one [128, 256] matmul per window computes
[T1(w) | T0(w+1)] = K_w vs [Q_w | Q_{w+1}] — 4 matmuls per 4-window group,
all 256-aligned (no PSUM bank crossing). Window w's prev-block T0(w) is
read from the previous group's P tile; the very first window reads a
constant zero tile (its prev window is fully masked padding).

Host-side shard prep (inside kernel(), numpy):
  qTp [2, 128, 8320]  — head-pair Q^T (head A rows 0-63, head B rows
                        64-127), one zero window appended (lookahead pad)
  kT  [2, 128, 8192]  — head-pair K^T
  vp  [4, 128, 4225]  — per head: 65 V slots x [V(64) | 1], slot 0 zeroed
  mask01 [128, 128]   — within-window causal 0/1 (k <= q)
Output:
  outT [4, 65, 8192]  — rows 0..63 unnormalized O^T, row 64 row-sums r
"""

import numpy as np

B, H, T, D = 4, 8, 8192, 64
W = 128                     # window size
WIN = T // W                # 64 windows per head
NCORES = 8
BH = B * H                  # 32
BH_PER_CORE = BH // NCORES  # 4
NPAIR = BH_PER_CORE // 2    # 2 head pairs per core
CHUNK_W = 64                # windows per load chunk
NCHUNK = WIN // CHUNK_W     # 2
G = 4                       # windows per softmax group (PSUM tile = [128, 1024])
SCALE = float(D) ** -0.5

MASK_ON_GPSIMD = True

_nc_cache = {}
last_perf = None


def _build_nc(skip=()):
    import concourse.tile as tile
    from concourse import bacc
    from concourse import mybir
    from contextlib import ExitStack

    f32 = mybir.dt.float32
    bf16 = mybir.dt.bfloat16
    Exp = mybir.ActivationFunctionType.Exp
    mult = mybir.AluOpType.mult

    nc = bacc.Bacc()
    qTp = nc.dram_tensor("qTp", [NPAIR, W, (WIN + 1) * W], bf16,
                         kind="ExternalInput")
    kT = nc.dram_tensor("kT", [NPAIR, W, T], bf16, kind="ExternalInput")
    vp = nc.dram_tensor("vp", [BH_PER_CORE, W, (WIN + 1) * (D + 1)], bf16,
                        kind="ExternalInput")
    mask = nc.dram_tensor("mask01", [W, W], bf16, kind="ExternalInput")
    outT = nc.dram_tensor("outT", [BH_PER_CORE, D + 1, T], bf16,
                          kind="ExternalOutput")

    with tile.TileContext(nc) as tc, ExitStack() as ctx:
        cpool = ctx.enter_context(tc.tile_pool(name="cpool", bufs=1))
        qpool = ctx.enter_context(tc.tile_pool(name="qpool", bufs=2))
        kpool = ctx.enter_context(tc.tile_pool(name="kpool", bufs=2))
        vpool = ctx.enter_context(tc.tile_pool(name="vpool", bufs=4))
        opool = ctx.enter_context(tc.tile_pool(name="opool", bufs=3))
        ppool = ctx.enter_context(tc.tile_pool(name="ppool", bufs=9))
        spsum = ctx.enter_context(tc.tile_pool(name="spsum", bufs=2, space="PSUM"))
        opsum = ctx.enter_context(tc.tile_pool(name="opsum", bufs=4, space="PSUM"))

        mtile = cpool.tile([W, W], bf16)
        nc.sync.dma_start(mtile[:], mask[:])
        z128 = cpool.tile([W, W], bf16)      # P for the all-masked pad window
        nc.vector.memset(z128[:], 0.0)

        mm = nc.tensor.matmul
        gidx = 0
        for p in range(NPAIR):
            # per-head P tile of the previous group (for cross-group T0 reads)
            pt_prev = [None, None]
            pending = [[], []]
            for c in range(NCHUNK):
                c0 = c * CHUNK_W * W
                qc = qpool.tile([W, (CHUNK_W + 1) * W], bf16, tag="qc")
                kc = kpool.tile([W, CHUNK_W * W], bf16, tag="kc")
                first = p == 0 and c == 0
                if "loads" not in skip:
                    if first:
                        # split so the first groups' data lands sooner
                        s = 5 * W
                        nc.sync.dma_start(qc[:, :s], qTp[p, :, c0:c0 + s])
                        nc.sync.dma_start(kc[:, :s], kT[p, :, c0:c0 + s])
                        nc.sync.dma_start(
                            qc[:, s:], qTp[p, :, c0 + s:c0 + (CHUNK_W + 1) * W])
                        nc.sync.dma_start(
                            kc[:, s:], kT[p, :, c0 + s:c0 + CHUNK_W * W])
                    else:
                        nc.sync.dma_start(
                            qc[:], qTp[p, :, c0:c0 + (CHUNK_W + 1) * W])
                        nc.sync.dma_start(kc[:], kT[p, :, c0:c0 + CHUNK_W * W])
                vcs = []
                ocs = []
                for h in range(2):
                    vc = vpool.tile([W, (CHUNK_W + 1) * (D + 1)], bf16, tag="vc")
                    if "loads" not in skip:
                        v0 = c * CHUNK_W * (D + 1)
                        sv = 5 * (D + 1)
                        if first:
                            nc.gpsimd.dma_start(
                                vc[:, :sv], vp[2 * p + h, :, v0:v0 + sv])
                            nc.gpsimd.dma_start(
                                vc[:, sv:],
                                vp[2 * p + h,
                                   :, v0 + sv:v0 + (CHUNK_W + 1) * (D + 1)])
                        else:
                            nc.gpsimd.dma_start(
                                vc[:],
                                vp[2 * p + h, :, v0:v0 + (CHUNK_W + 1) * (D + 1)])
                    vcs.append(vc)
                    oc = opool.tile([D + 1, CHUNK_W * W], bf16, tag="oc")
                    ocs.append(oc)

                def emit_pv(st):
                    # PV + row-sums for a finished group, 5 matmuls:
                    #   A: slot w0+1 x pb(w0) [256w, start] -> cols w0,w1
                    #   B: slot w0+3 x pb(w2) [256w, start] -> cols w2,w3
                    #   C: slot w0+2 x pb(w1) [256w, accum] -> cols w1,w2
                    #   D: slot w0   x T0(w0) [128w, accum] -> cols w0
                    #   E: slot w0+4 x T1(w3) [128w, accum] -> cols w3
                    g, h, pt, prev, vc, oc = st
                    w0 = g * G
                    vs = lambda s: vc[:, s * (D + 1):(s + 1) * (D + 1)]
                    t0 = prev[:, G * 256 - W:G * 256] if prev is not None \
                        else z128[:]
                    op = opsum.tile([D + 1, G * W], f32, tag="op")
                    if "pv" not in skip:
                        # A's start=True marks the whole 2KB bank pending-zero;
                        # B's cols are still pending when it writes, so it
                        # overwrites (acts as its own start). C/D/E touch only
                        # bytes already written by A/B, so they accumulate.
                        mm(op[:, 0:2 * W], vs(w0 + 1), pt[:, 0:256],
                           start=True, stop=True)
                        mm(op[:, 2 * W:4 * W], vs(w0 + 3), pt[:, 512:768],
                           start=False, stop=False, skip_group_check=True)
                        mm(op[:, W:3 * W], vs(w0 + 2), pt[:, 256:512],
                           start=False, stop=False, skip_group_check=True)
                        mm(op[:, 0:W], vs(w0), t0,
                           start=False, stop=False, skip_group_check=True)
                        mm(op[:, 3 * W:4 * W], vs(w0 + 4), pt[:, 768:768 + W],
                           start=False, stop=True, skip_group_check=True)
                    if "ocopy" not in skip:
                        nc.vector.tensor_copy(
                            oc[:, w0 * W:(w0 + G) * W], op[:])

                for g in range(CHUNK_W // G):
                    w0 = g * G
                    for h in range(2):
                        hb = h * 64  # partition base of this head in qc/kc
                        # pairblock matmuls: [T1(w) | T0(w+1)] per window
                        sp = spsum.tile([W, G * 2 * W], f32, tag="sp")
                        if "smm" not in skip:
                            for i in range(G):
                                wl = w0 + i
                                mm(sp[:, i * 256:(i + 1) * 256],
                                   kc[hb:hb + 64, wl * W:(wl + 1) * W],
                                   qc[hb:hb + 64, wl * W:(wl + 2) * W],
                                   start=True, stop=True)

                        pt = ppool.tile([W, G * 2 * W], bf16, tag="pt")
                        if "exp" not in skip:
                            nc.scalar.activation(pt[:], sp[:], Exp, scale=SCALE)

                        # causal mask on T1 blocks (cols 0,256,512,768)
                        pt3 = pt[:].rearrange("p (g x) -> p g x", x=2 * W)
                        t1 = pt3[:, :, 0:W]
                        mb = mtile[:, None, :].to_broadcast([W, G, W])
                        if "mask" not in skip:
                            eng = nc.gpsimd if h == 0 else nc.vector
                            eng.tensor_tensor(t1, t1, mb, mult)

                        # PV lags two group-heads so exp+mask are off the
                        # tensor engine's critical path (in-order queue)
                        if len(pending[h]) == 1:
                            emit_pv(pending[h].pop(0))
                        pending[h].append((g, h, pt, pt_prev[h], vcs[h], ocs[h]))
                        pt_prev[h] = pt

                    # store finished pieces early to overlap the tail
                    last = p == NPAIR - 1 and c == NCHUNK - 1
                    ng = CHUNK_W // G
                    qtr = CHUNK_W * W // 4
                    if "store" not in skip:
                        if not last and g == ng // 2 + 1:
                            for h in range(2):
                                nc.gpsimd.dma_start(
                                    outT[2 * p + h, :, c0:c0 + 2 * qtr],
                                    ocs[h][:, :2 * qtr])
                        elif last and g in (ng // 4 + 1, ng // 2 + 1,
                                            3 * ng // 4 + 1):
                            q0 = (g - ng // 4 - 1) // (ng // 4) * qtr
                            for h in range(2):
                                nc.gpsimd.dma_start(
                                    outT[2 * p + h, :, c0 + q0:c0 + q0 + qtr],
                                    ocs[h][:, q0:q0 + qtr])

                # flush pending PV before switching chunks so ocs/vcs of
                # this chunk complete, then store the rest
                for h in range(2):
                    while pending[h]:
                        emit_pv(pending[h].pop(0))
                if "store" not in skip:
                    half = CHUNK_W * W // 2
                    s0_ = half if not (p == NPAIR - 1 and c == NCHUNK - 1) \
                        else 3 * (CHUNK_W * W // 4)
                    for h in range(2):
                        nc.gpsimd.dma_start(
                            outT[2 * p + h, :, c0 + s0_:c0 + CHUNK_W * W],
                            ocs[h][:, s0_:])
    nc.finalize()
    return nc


def _prep_core_inputs(q2, k2, v2, core):
    from ml_dtypes import bfloat16
    s0 = core * BH_PER_CORE
    qTp = np.zeros((NPAIR, W, (WIN + 1) * W), bfloat16)
    kTp = np.zeros((NPAIR, W, T), bfloat16)
    for p in range(NPAIR):
        for h in range(2):
            bh = s0 + 2 * p + h
            qTp[p, h * 64:(h + 1) * 64, :T] = q2[bh].T
            kTp[p, h * 64:(h + 1) * 64, :] = k2[bh].T
    vr = v2[s0:s0 + BH_PER_CORE].reshape(
        BH_PER_CORE, WIN, W, D).transpose(0, 2, 1, 3)
    vp = np.zeros((BH_PER_CORE, W, WIN + 1, D + 1), bfloat16)
    vp[:, :, 1:, :D] = vr
    vp[:, :, :, D] = 1.0
    vp = np.ascontiguousarray(vp.reshape(BH_PER_CORE, W, (WIN + 1) * (D + 1)))
    mask01 = (np.arange(W)[:, None] <= np.arange(W)[None, :]).astype(bfloat16)
    return {"qTp": qTp, "kT": kTp, "vp": vp, "mask01": mask01}


def kernel(q, k, v, _trace=False):
    global last_perf
    from concourse.bass_utils import run_bass_kernel_spmd

    q = np.ascontiguousarray(np.asarray(q), dtype=np.float32)
    k = np.ascontiguousarray(np.asarray(k), dtype=np.float32)
    v = np.ascontiguousarray(np.asarray(v), dtype=np.float32)
    q2 = q.reshape(BH, T, D)
    k2 = k.reshape(BH, T, D)
    v2 = v.reshape(BH, T, D)

    if "nc" not in _nc_cache:
        _nc_cache["nc"] = _build_nc()
    nc = _nc_cache["nc"]

    in_maps = [_prep_core_inputs(q2, k2, v2, core) for core in range(NCORES)]
    res = run_bass_kernel_spmd(
        nc, in_maps, core_ids=list(range(NCORES)), trace=_trace)
    last_perf = res

    outs = []
    for core in range(NCORES):
        ot = np.asarray(res.results[core]["outT"], np.float32)  # [4, 65, T]
        o = ot[:, :D, :] / ot[:, D:D + 1, :]           # normalize
        outs.append(o.transpose(0, 2, 1))              # [4, T, 64]
    full = np.concatenate(outs, axis=0)                # [32, T, 64]
    return full.reshape(B, H, T, D)



# revision 18
# speedup vs baseline: 1.0430x; 1.0430x over previous
"""LocalAttention Bass kernel for Trainium2 (8 NeuronCores).

Problem: B=4 H=8 T=8192 D=64, window=128, look_backward=1, causal.
Sharding: pure (B*H) data parallelism — 32 heads / 8 cores = 4 heads each,
processed as 2 head-pairs so q/k DMAs use all 128 SBUF partitions.

Device algorithm (per head, per 128-token window w):
  S^T[k, q] = K_w' @ Q_w^T      (keys on partitions, so the softmax
                                 reduction over keys can ride the PV matmul)
  P = exp(S^T * D^-0.5) * causal01
  [O^T; r] = [V | 1]^T @ P      (ones column baked into V gives row-sums)
Host divides O^T by r and transposes back.

PSUM "pairblock" layout: one [128, 256] matmul per window computes
[T1(w) | T0(w+1)] = K_w vs [Q_w | Q_{w+1}] — 4 matmuls per 4-window group,
all 256-aligned (no PSUM bank crossing). Window w's prev-block T0(w) is
read from the previous group's P tile; the very first window reads a
constant zero tile (its prev window is fully masked padding).

Host-side shard prep (inside kernel(), numpy):
  qTp [2, 128, 8320]  — head-pair Q^T (head A rows 0-63, head B rows
                        64-127), one zero window appended (lookahead pad)
  kT  [2, 128, 8192]  — head-pair K^T
  vp  [4, 128, 4225]  — per head: 65 V slots x [V(64) | 1], slot 0 zeroed
  mask01 [128, 128]   — within-window causal 0/1 (k <= q)
Output:
  outT [4, 65, 8192]  — rows 0..63 unnormalized O^T, row 64 row-sums r
"""

import numpy as np

B, H, T, D = 4, 8, 8192, 64
W = 128                     # window size
WIN = T // W                # 64 windows per head
NCORES = 8
BH = B * H                  # 32
BH_PER_CORE = BH // NCORES  # 4
NPAIR = BH_PER_CORE // 2    # 2 head pairs per core
CHUNK_W = 32                # windows per load chunk
NCHUNK = WIN // CHUNK_W     # 2
G = 4                       # windows per softmax group (PSUM tile = [128, 1024])
SCALE = float(D) ** -0.5

MASK_ON_GPSIMD = True

_nc_cache = {}
last_perf = None


def _build_nc(skip=()):
    import concourse.tile as tile
    from concourse import bacc
    from concourse import mybir
    from contextlib import ExitStack

    f32 = mybir.dt.float32
    bf16 = mybir.dt.bfloat16
    Exp = mybir.ActivationFunctionType.Exp
    mult = mybir.AluOpType.mult

    nc = bacc.Bacc()
    qTp = nc.dram_tensor("qTp", [NPAIR, W, (WIN + 1) * W], bf16,
                         kind="ExternalInput")
    kT = nc.dram_tensor("kT", [NPAIR, W, T], bf16, kind="ExternalInput")
    vp = nc.dram_tensor("vp", [BH_PER_CORE, W, (WIN + 1) * (D + 1)], bf16,
                        kind="ExternalInput")
    mask = nc.dram_tensor("mask01", [W, W], bf16, kind="ExternalInput")
    outT = nc.dram_tensor("outT", [BH_PER_CORE, D + 1, T], bf16,
                          kind="ExternalOutput")

    with tile.TileContext(nc) as tc, ExitStack() as ctx:
        cpool = ctx.enter_context(tc.tile_pool(name="cpool", bufs=1))
        qpool = ctx.enter_context(tc.tile_pool(name="qpool", bufs=2))
        kpool = ctx.enter_context(tc.tile_pool(name="kpool", bufs=2))
        vpool = ctx.enter_context(tc.tile_pool(name="vpool", bufs=4))
        opool = ctx.enter_context(tc.tile_pool(name="opool", bufs=3))
        ppool = ctx.enter_context(tc.tile_pool(name="ppool", bufs=9))
        spsum = ctx.enter_context(tc.tile_pool(name="spsum", bufs=2, space="PSUM"))
        opsum = ctx.enter_context(tc.tile_pool(name="opsum", bufs=4, space="PSUM"))

        mtile = cpool.tile([W, W], bf16)
        nc.sync.dma_start(mtile[:], mask[:])
        z128 = cpool.tile([W, W], bf16)      # P for the all-masked pad window
        nc.vector.memset(z128[:], 0.0)

        mm = nc.tensor.matmul
        gidx = 0
        for p in range(NPAIR):
            # per-head P tile of the previous group (for cross-group T0 reads)
            pt_prev = [None, None]
            pending = [[], []]
            for c in range(NCHUNK):
                c0 = c * CHUNK_W * W
                qc = qpool.tile([W, (CHUNK_W + 1) * W], bf16, tag="qc")
                kc = kpool.tile([W, CHUNK_W * W], bf16, tag="kc")
                first = p == 0 and c == 0
                if "loads" not in skip:
                    if first:
                        # split so the first groups' data lands sooner
                        s = 5 * W
                        nc.sync.dma_start(qc[:, :s], qTp[p, :, c0:c0 + s])
                        nc.sync.dma_start(kc[:, :s], kT[p, :, c0:c0 + s])
                        nc.sync.dma_start(
                            qc[:, s:], qTp[p, :, c0 + s:c0 + (CHUNK_W + 1) * W])
                        nc.sync.dma_start(
                            kc[:, s:], kT[p, :, c0 + s:c0 + CHUNK_W * W])
                    else:
                        nc.sync.dma_start(
                            qc[:], qTp[p, :, c0:c0 + (CHUNK_W + 1) * W])
                        nc.sync.dma_start(kc[:], kT[p, :, c0:c0 + CHUNK_W * W])
                vcs = []
                ocs = []
                for h in range(2):
                    vc = vpool.tile([W, (CHUNK_W + 1) * (D + 1)], bf16, tag="vc")
                    if "loads" not in skip:
                        v0 = c * CHUNK_W * (D + 1)
                        sv = 5 * (D + 1)
                        if first:
                            nc.gpsimd.dma_start(
                                vc[:, :sv], vp[2 * p + h, :, v0:v0 + sv])
                            nc.gpsimd.dma_start(
                                vc[:, sv:],
                                vp[2 * p + h,
                                   :, v0 + sv:v0 + (CHUNK_W + 1) * (D + 1)])
                        else:
                            nc.gpsimd.dma_start(
                                vc[:],
                                vp[2 * p + h, :, v0:v0 + (CHUNK_W + 1) * (D + 1)])
                    vcs.append(vc)
                    oc = opool.tile([D + 1, CHUNK_W * W], bf16, tag="oc")
                    ocs.append(oc)

                def emit_pv(st):
                    # PV + row-sums for a finished group, 5 matmuls:
                    #   A: slot w0+1 x pb(w0) [256w, start] -> cols w0,w1
                    #   B: slot w0+3 x pb(w2) [256w, start] -> cols w2,w3
                    #   C: slot w0+2 x pb(w1) [256w, accum] -> cols w1,w2
                    #   D: slot w0   x T0(w0) [128w, accum] -> cols w0
                    #   E: slot w0+4 x T1(w3) [128w, accum] -> cols w3
                    g, h, pt, prev, vc, oc = st
                    w0 = g * G
                    vs = lambda s: vc[:, s * (D + 1):(s + 1) * (D + 1)]
                    t0 = prev[:, G * 256 - W:G * 256] if prev is not None \
                        else z128[:]
                    op = opsum.tile([D + 1, G * W], f32, tag="op")
                    if "pv" not in skip:
                        # A's start=True marks the whole 2KB bank pending-zero;
                        # B's cols are still pending when it writes, so it
                        # overwrites (acts as its own start). C/D/E touch only
                        # bytes already written by A/B, so they accumulate.
                        mm(op[:, 0:2 * W], vs(w0 + 1), pt[:, 0:256],
                           start=True, stop=True)
                        mm(op[:, 2 * W:4 * W], vs(w0 + 3), pt[:, 512:768],
                           start=False, stop=False, skip_group_check=True)
                        mm(op[:, W:3 * W], vs(w0 + 2), pt[:, 256:512],
                           start=False, stop=False, skip_group_check=True)
                        mm(op[:, 0:W], vs(w0), t0,
                           start=False, stop=False, skip_group_check=True)
                        mm(op[:, 3 * W:4 * W], vs(w0 + 4), pt[:, 768:768 + W],
                           start=False, stop=True, skip_group_check=True)
                    if "ocopy" not in skip:
                        nc.vector.tensor_copy(
                            oc[:, w0 * W:(w0 + G) * W], op[:])

                for g in range(CHUNK_W // G):
                    w0 = g * G
                    for h in range(2):
                        hb = h * 64  # partition base of this head in qc/kc
                        # pairblock matmuls: [T1(w) | T0(w+1)] per window
                        sp = spsum.tile([W, G * 2 * W], f32, tag="sp")
                        if "smm" not in skip:
                            for i in range(G):
                                wl = w0 + i
                                mm(sp[:, i * 256:(i + 1) * 256],
                                   kc[hb:hb + 64, wl * W:(wl + 1) * W],
                                   qc[hb:hb + 64, wl * W:(wl + 2) * W],
                                   start=True, stop=True)

                        pt = ppool.tile([W, G * 2 * W], bf16, tag="pt")
                        if "exp" not in skip:
                            nc.scalar.activation(pt[:], sp[:], Exp, scale=SCALE)

                        # causal mask on T1 blocks (cols 0,256,512,768)
                        pt3 = pt[:].rearrange("p (g x) -> p g x", x=2 * W)
                        t1 = pt3[:, :, 0:W]
                        mb = mtile[:, None, :].to_broadcast([W, G, W])
                        if "mask" not in skip:
                            eng = nc.gpsimd if h == 0 else nc.vector
                            eng.tensor_tensor(t1, t1, mb, mult)

                        # PV lags two group-heads so exp+mask are off the
                        # tensor engine's critical path (in-order queue)
                        if len(pending[h]) == 1:
                            emit_pv(pending[h].pop(0))
                        pending[h].append((g, h, pt, pt_prev[h], vcs[h], ocs[h]))
                        pt_prev[h] = pt

                    # store finished pieces early to overlap the tail
                    last = p == NPAIR - 1 and c == NCHUNK - 1
                    ng = CHUNK_W // G
                    qtr = CHUNK_W * W // 4
                    if "store" not in skip:
                        if not last and g == ng // 2 + 1:
                            for h in range(2):
                                nc.gpsimd.dma_start(
                                    outT[2 * p + h, :, c0:c0 + 2 * qtr],
                                    ocs[h][:, :2 * qtr])
                        elif last and g in (ng // 4 + 1, ng // 2 + 1,
                                            3 * ng // 4 + 1):
                            q0 = (g - ng // 4 - 1) // (ng // 4) * qtr
                            for h in range(2):
                                nc.gpsimd.dma_start(
                                    outT[2 * p + h, :, c0 + q0:c0 + q0 + qtr],
                                    ocs[h][:, q0:q0 + qtr])

                # flush pending PV before switching chunks so ocs/vcs of
                # this chunk complete, then store the rest
                for h in range(2):
                    while pending[h]:
                        emit_pv(pending[h].pop(0))
                if "store" not in skip:
                    half = CHUNK_W * W // 2
                    s0_ = half if not (p == NPAIR - 1 and c == NCHUNK - 1) \
                        else 3 * (CHUNK_W * W // 4)
                    for h in range(2):
                        nc.gpsimd.dma_start(
                            outT[2 * p + h, :, c0 + s0_:c0 + CHUNK_W * W],
                            ocs[h][:, s0_:])
    nc.finalize()
    return nc


def _prep_core_inputs(q2, k2, v2, core):
    from ml_dtypes import bfloat16
    s0 = core * BH_PER_CORE
    qTp = np.zeros((NPAIR, W, (WIN + 1) * W), bfloat16)
    kTp = np.zeros((NPAIR, W, T), bfloat16)
    for p in range(NPAIR):
        for h in range(2):
            bh = s0 + 2 * p + h
            qTp[p, h * 64:(h + 1) * 64, :T] = q2[bh].T
            kTp[p, h * 64:(h + 1) * 64, :] = k2[bh].T
    vr = v2[s0:s0 + BH_PER_CORE].reshape(
        BH_PER_CORE, WIN, W, D).transpose(0, 2, 1, 3)
    vp = np.zeros((BH_PER_CORE, W, WIN + 1, D + 1), bfloat16)
    vp[:, :, 1:, :D] = vr
    vp[:, :, :, D] = 1.0
    vp = np.ascontiguousarray(vp.reshape(BH_PER_CORE, W, (WIN + 1) * (D + 1)))
    mask01 = (np.arange(W)[:, None] <= np.arange(W)[None, :]).astype(bfloat16)
    return {"qTp": qTp, "kT": kTp, "vp": vp, "mask01": mask01}


def kernel(q, k, v, _trace=False):
    global last_perf
    from concourse.bass_utils import run_bass_kernel_spmd

    q = np.ascontiguousarray(np.asarray(q), dtype=np.float32)
    k = np.ascontiguousarray(np.asarray(k), dtype=np.float32)
    v = np.ascontiguousarray(np.asarray(v), dtype=np.float32)
    q2 = q.reshape(BH, T, D)
    k2 = k.reshape(BH, T, D)
    v2 = v.reshape(BH, T, D)

    if "nc" not in _nc_cache:
        _nc_cache["nc"] = _build_nc()
    nc = _nc_cache["nc"]

    in_maps = [_prep_core_inputs(q2, k2, v2, core) for core in range(NCORES)]
    res = run_bass_kernel_spmd(
        nc, in_maps, core_ids=list(range(NCORES)), trace=_trace)
    last_perf = res

    outs = []
    for core in range(NCORES):
        ot = np.asarray(res.results[core]["outT"], np.float32)  # [4, 65, T]
        o = ot[:, :D, :] / ot[:, D:D + 1, :]           # normalize
        outs.append(o.transpose(0, 2, 1))              # [4, T, 64]
    full = np.concatenate(outs, axis=0)                # [32, T, 64]
    return full.reshape(B, H, T, D)

